# revision 7
# baseline (speedup 1.0000x reference)
"""Single-query attention with double softmax on 8 TRN2 NeuronCores.

Problem: q [32,512], k [32,8192,512], v [32,8192,512] (all fp32)
  scores = einsum("bd,bld->bl", q, k)
  att    = softmax(softmax(scores))          (over l, twice)
  ctx    = einsum("bl,bld->bd", att, v)
Returns (ctx [32,512], att [32,1,8192]).

Sharding: pure data parallel — batch dim split 4-per-core across 8 cores.

Per-core kernel, per batch element:
  - q broadcast to 128 partitions via gpsimd.partition_broadcast.
  - QK: k streamed as [128p, 4, 512] tiles (1 MiB contiguous DMAs);
    fused DVE tensor_tensor_reduce gives scores[p, c] for l = c*128+p.
  - softmax: per-partition stats on DVE/ACT (fused exp+sum), cross-
    partition combine via gpsimd.partition_all_reduce; second softmax
    needs no max (inputs in [0,1]); both normalizations fold into ACT
    activation scale operands.
  - AV: 64 accumulating PE matmuls, att column [128,1] stationary,
    v chunk [128,512] moving, into a [1,512] PSUM accumulator; the
    1/Z2 normalization rides the ACT PSUM->SBUF output copies.

Hardware constraint honored throughout: a PE matmul can carry at most
ONE semaphore wait, so PE reads either ACT-produced tiles (covered by
one ACT wait) or DMA tiles (one DMA-queue wait), never a mix of new
semaphores; a priming transpose at init absorbs the gpsimd identity
dependency once.
"""

import numpy as np

B, L, D = 32, 8192, 512
N_CORES = 8
BPC = B // N_CORES  # batches per core
TPW = 4             # 128-row chunks per DMA tile
NT = L // (128 * TPW)   # DMA tiles per batch (16)
NCH = L // 128          # 128-row chunks per batch (64)

_cached = {}


def _build_nc():
    import concourse.bacc as bacc
    import concourse.mybir as mybir
    from concourse import bass_isa, library_config
    from concourse._compat import get_trn_type
    from concourse.tile import TileContext
    from concourse.masks import make_identity

    fp32 = mybir.dt.float32
    AF = mybir.ActivationFunctionType
    ALU = mybir.AluOpType
    AX = mybir.AxisListType
    ROp = bass_isa.ReduceOp

    nc = bacc.Bacc(
        get_trn_type() or "TRN2",
        target_bir_lowering=False,
        debug=False,
        enable_asserts=True,
    )
    q_d = nc.dram_tensor("q", [BPC, D], fp32, kind="ExternalInput")
    k_d = nc.dram_tensor("k", [BPC, L, D], fp32, kind="ExternalInput")
    v_d = nc.dram_tensor("v", [BPC, L, D], fp32, kind="ExternalInput")
    ctx_d = nc.dram_tensor("ctx", [BPC, D], fp32, kind="ExternalOutput")
    att_d = nc.dram_tensor("att", [BPC, L], fp32, kind="ExternalOutput")

    # l = (T*TPW + t)*128 + p  for tile T, chunk t, partition p
    k_r = k_d.rearrange("b (T t p) d -> b T p t d", t=TPW, p=128)
    v_r = v_d.rearrange("b (T t p) d -> b T p t d", t=TPW, p=128)
    att_r = att_d.rearrange("b (c p) -> b c p", p=128)

    with TileContext(nc) as tc:
        with (
            tc.tile_pool(name="consts", bufs=1) as consts,
            tc.tile_pool(name="qrow", bufs=2) as qrow_p,
            tc.tile_pool(name="qbc", bufs=2) as qbc_p,
            tc.tile_pool(name="kp", bufs=3) as kp,
            tc.tile_pool(name="vp", bufs=4) as vp,
            tc.tile_pool(name="work", bufs=2) as work,
            tc.tile_pool(name="small", bufs=2) as small,
            tc.tile_pool(name="ps_sm", bufs=3, space="PSUM") as ps_sm,
            tc.tile_pool(name="ps_ctx", bufs=3, space="PSUM") as ps_ctx,
            tc.tile_pool(name="ps_prime", bufs=1, space="PSUM") as ps_prime,
        ):
            ident = consts.tile([128, 128], fp32)
            make_identity(nc, ident[:])
            # partition_{broadcast,all_reduce} live in the attn ucode library
            nc.gpsimd.load_library(library_config.attn)
            # Prime PE's gpsimd vector clock on the identity once, so the
            # per-batch transposes never need a second (gpsimd) wait.
            prime_ps = ps_prime.tile([128, 128], fp32)
            nc.tensor.transpose(prime_ps[:], ident[:], ident[:])

            for b in range(BPC):
                # ---- q load (replicated TPW x in free) + partition bcast ----
                q_sb = qrow_p.tile([1, TPW * D], fp32, tag="q_sb")
                for t in range(TPW):
                    nc.sync.dma_start(
                        out=q_sb[0:1, t * D : (t + 1) * D], in_=q_d[b : b + 1, :]
                    )
                q_bc = qbc_p.tile([128, TPW * D], fp32, tag="q_bc")
                nc.gpsimd.partition_broadcast(q_bc[:], q_sb[:], channels=128)

                # ---- QK^T: scores[p, c] = sum_d k[l=c*128+p, d] * q[d] ----
                # DVE does the elementwise product on the whole 1MiB tile;
                # ACT accumulates each 512-chunk into scores via accum_out.
                scores = work.tile([128, NCH], fp32, tag="scores")
                for T in range(NT):
                    k_t = kp.tile([128, TPW, D], fp32, tag="k_t")
                    nc.sync.dma_start(out=k_t[:], in_=k_r[b, T])
                    y = work.tile([128, TPW * D], fp32, tag="qk_y")
                    nc.vector.tensor_tensor(
                        y[:],
                        k_t[:].rearrange("p t d -> p (t d)"),
                        q_bc[:],
                        ALU.mult,
                    )
                    for t in range(TPW):
                        c = T * TPW + t
                        scr = work.tile([128, D], fp32, tag="qk_scr")
                        nc.scalar.activation(
                            out=scr[:],
                            in_=y[:, t * D : (t + 1) * D],
                            func=AF.Copy,
                            accum_out=scores[:, c : c + 1],
                        )

                # ---- softmax #1 (exact, global over 8192) ----
                m_p = small.tile([128, 1], fp32, tag="m_p")
                nc.vector.tensor_reduce(
                    out=m_p[:], in_=scores[:], axis=AX.X, op=ALU.max,
                )
                m_all = small.tile([128, 1], fp32, tag="m_all")
                nc.gpsimd.partition_all_reduce(
                    m_all[:], m_p[:], channels=128, reduce_op=ROp.max,
                )
                negM = small.tile([128, 1], fp32, tag="negM")
                nc.vector.tensor_scalar_mul(negM[:], m_all[:], -1.0)
                exp1 = work.tile([128, NCH], fp32, tag="exp1")
                Z_p = small.tile([128, 1], fp32, tag="Z_p")
                nc.scalar.activation(
                    out=exp1[:], in_=scores[:], func=AF.Exp,
                    bias=negM[:, 0:1], scale=1.0, accum_out=Z_p[:],
                )  # exp1 = exp(s - M); Z_p = row sums
                Z_all = small.tile([128, 1], fp32, tag="Z_all")
                nc.gpsimd.partition_all_reduce(
                    Z_all[:], Z_p[:], channels=128, reduce_op=ROp.add,
                )
                Zr = small.tile([128, 1], fp32, tag="Zr")
                nc.vector.reciprocal(Zr[:], Z_all[:])

                # ---- softmax #2: att = exp(exp1/Z) / Z2 (no max needed) ----
                exp2 = work.tile([128, NCH], fp32, tag="exp2")
                Z2_p = small.tile([128, 1], fp32, tag="Z2_p")
                nc.scalar.activation(
                    out=exp2[:], in_=exp1[:], func=AF.Exp,
                    bias=0.0, scale=Zr[:, 0:1], accum_out=Z2_p[:],
                )
                Z2_all = small.tile([128, 1], fp32, tag="Z2_all")
                nc.gpsimd.partition_all_reduce(
                    Z2_all[:], Z2_p[:], channels=128, reduce_op=ROp.add,
                )
                Z2r = small.tile([128, 1], fp32, tag="Z2r")
                nc.vector.reciprocal(Z2r[:], Z2_all[:])

                # ---- att output: PE transpose exp2 -> [64c,128p], ACT copy
                #      applies the 1/Z2 scale on the way out ----
                attT_ps = ps_sm.tile([64, 128], fp32, tag="attT_ps")
                nc.tensor.transpose(attT_ps[:], exp2[:], ident[:])
                attT = work.tile([64, 128], fp32, tag="attT")
                nc.scalar.activation(
                    out=attT[:], in_=attT_ps[:], func=AF.Copy,
                    scale=Z2r[0:64, 0:1],
                )
                nc.sync.dma_start(out=att_r[b], in_=attT[:])

                # ---- AV: ctx_raw[1,512] += exp2[:,c].T @ v_chunk ----
                ctx_ps = ps_ctx.tile([1, D], fp32, tag="ctx_ps")
                for T in range(NT):
                    v_t = vp.tile([128, TPW, D], fp32, tag="v_t")
                    nc.sync.dma_start(out=v_t[:], in_=v_r[b, T])
                    for t in range(TPW):
                        c = T * TPW + t
                        nc.tensor.matmul(
                            ctx_ps[:], exp2[:, c : c + 1], v_t[:, t, :],
                            start=(c == 0), stop=(c == NCH - 1),
                        )
                ctx_sb = small.tile([1, D], fp32, tag="ctx_sb")
                nc.scalar.activation(
                    out=ctx_sb[:], in_=ctx_ps[:], func=AF.Copy,
                    scale=Z2r[0:1, 0:1],
                )
                nc.sync.dma_start(out=ctx_d[b : b + 1, :], in_=ctx_sb[:])

    nc.finalize()
    return nc


def kernel(q, k, v):
    from concourse.bass_utils import run_bass_kernel_spmd

    if "nc" not in _cached:
        _cached["nc"] = _build_nc()
    nc = _cached["nc"]

    q = np.asarray(q, dtype=np.float32)
    k = np.asarray(k, dtype=np.float32)
    v = np.asarray(v, dtype=np.float32)

    in_maps = []
    for c in range(N_CORES):
        s = slice(c * BPC, (c + 1) * BPC)
        in_maps.append({"q": q[s], "k": k[s], "v": v[s]})

    res = run_bass_kernel_spmd(nc, in_maps, core_ids=list(range(N_CORES)))
    ctx = np.concatenate([r["ctx"] for r in res.results], axis=0)
    att = np.concatenate([r["att"] for r in res.results], axis=0)
    return ctx, att.reshape(B, 1, L)
